# revision 8
# baseline (speedup 1.0000x reference)
"""Cumulative mean along T (running mean) for input [8, 4096, 1024] f32.

out[b, t, f] = mean(x[b, :t+1, f])

Pure data parallel over batch: 8 cores, one batch element each.
Per core, blocked prefix-sum along T in 128-row blocks:

  - Rotated-output triangular matmul per block (stationary col 0 = ones,
    col t>=1 ones for s <= t-1): PSUM partition 0 = block total (a legal AP
    base), partitions 1..127 = local prefixes of rows 0..126.
  - Main matmuls are carry-FREE and independent across blocks (exact fp32).
  - The running carry is maintained as a [1, F] SBUF row on partition 0:
    carry_{i+1} = carry_i + psum_i[0:1] (one small DVE add per block - the
    only serial chain, ~0.3us per hop).
  - Carry applied to blocks i>0 by a K=1 ones-broadcast matmul accumulating
    into the same PSUM bank (float32r: full-rate; carry rows are produced
    f32r-rounded by the DVE adds to satisfy the verifier).
  - Per-row 1/(t+1) scale via tensor_scalar with a rotated per-partition
    reciprocal column.

DMA: inputs via HWDGE (nc.sync) which spreads DRAM->SBUF across all 16 SDMA
engines; outputs via SWDGE (nc.gpsimd) which spreads SBUF->DRAM (HWDGE does
not), batched 4 blocks (~2 MiB) per dma_start to amortize Q7 emission.
"""

import numpy as np

import concourse.bacc as bacc
import concourse.tile as tile
from concourse import mybir
from concourse.bass_utils import run_bass_kernel_spmd

B, T, F = 8, 4096, 1024
P = 128
NBLK = T // P  # 32
FH = 512       # one PSUM bank of f32
NHALF = F // FH
CPG = 4        # blocks per DMA group

F32 = mybir.dt.float32
F32R = mybir.dt.float32r


def _build():
    nc = bacc.Bacc(None, target_bir_lowering=False)
    x_dram = nc.dram_tensor("x", [T, F], F32, kind="ExternalInput")
    out_dram = nc.dram_tensor("out", [T, F], F32, kind="ExternalOutput")

    # Rotated prefix-sum stationary: col 0 all ones, col t>=1 strict-upper.
    lt_np = np.triu(np.ones((P, P), dtype=np.float32), 1)
    lt_np[:, 0] = 1.0
    # psum partition p of block i holds output row r: p=0 -> r=i*128+127,
    # p>=1 -> r=i*128+p-1. scale = 1/(r+1).
    rows = np.arange(T, dtype=np.int64).reshape(NBLK, P)
    r_of_p = np.empty((NBLK, P), dtype=np.int64)
    r_of_p[:, 0] = rows[:, P - 1]
    r_of_p[:, 1:] = rows[:, : P - 1]
    recip_np = np.ascontiguousarray((1.0 / (r_of_p.T + 1.0)).astype(np.float32))
    lt_dram = nc.inline_tensor(lt_np, "lt_const")
    recip_dram = nc.inline_tensor(recip_np, "recip_const")

    x_rot = x_dram.rearrange("(n p) f -> p n f", p=P)
    out_rot = out_dram.rearrange("(n p) f -> p n f", p=P)

    with tile.TileContext(nc) as tc:
        with (
            tc.tile_pool(name="const", bufs=1) as cpool,
            tc.tile_pool(name="xin", bufs=3) as xpool,
            tc.tile_pool(name="xout", bufs=3) as opool,
            tc.tile_pool(name="run", bufs=4) as rpool,
            tc.tile_pool(name="psum", bufs=4, space="PSUM") as ppool,
        ):
            lt = cpool.tile([P, P], F32)
            nc.sync.dma_start(lt[:], lt_dram[:])
            recip = cpool.tile([P, NBLK], F32)
            nc.sync.dma_start(recip[:], recip_dram[:])
            ones1_f32 = cpool.tile([1, P], F32)
            nc.vector.memset(ones1_f32[:], 1.0)
            ones1 = cpool.tile([1, P], F32R)
            nc.vector.tensor_copy(ones1[:], ones1_f32[:])

            carry = None  # [1, F] f32r SBUF row: sum of all raw blocks < i
            for g in range(NBLK // CPG):
                xt = xpool.tile([P, CPG, F], F32)
                nc.sync.dma_start(xt[:], x_rot[:, g * CPG : (g + 1) * CPG, :])
                ot = opool.tile([P, CPG, F], F32)

                psums = []
                carries = []
                # Main matmuls (carry-free, independent) + the carry chain.
                for c in range(CPG):
                    i = g * CPG + c
                    ps = ppool.tile([P, F], F32)
                    psums.append(ps)
                    carries.append(carry)
                    for h in range(NHALF):
                        nc.tensor.matmul(
                            ps[:, h * FH : (h + 1) * FH],
                            lt[:],
                            xt[:, c, h * FH : (h + 1) * FH],
                            start=True,
                            stop=(i == 0),
                        )
                    # Chain hop: next carry = carry + this block's raw total
                    # (psum row 0 before the broadcast matmul adds carry).
                    if i < NBLK - 1:
                        new_carry = rpool.tile([1, F], F32R)
                        if carry is None:
                            nc.vector.tensor_copy(new_carry[:], ps[0:1, :])
                        else:
                            nc.vector.tensor_tensor(
                                new_carry[:],
                                carry[:].bitcast(F32),
                                ps[0:1, :],
                                mybir.AluOpType.add,
                            )
                        carry = new_carry

                # Carry broadcast matmuls (K=1 ones, f32r full-rate).
                for c in range(CPG):
                    i = g * CPG + c
                    if carries[c] is None:
                        continue
                    ps = psums[c]
                    for h in range(NHALF):
                        nc.tensor.matmul(
                            ps[:, h * FH : (h + 1) * FH],
                            ones1[:],
                            carries[c][:, h * FH : (h + 1) * FH],
                            start=False,
                            stop=True,
                        )

                # Scale and emit.
                for c in range(CPG):
                    i = g * CPG + c
                    nc.vector.tensor_scalar(
                        ot[:, c, :],
                        psums[c][:],
                        recip[:, i : i + 1],
                        None,
                        mybir.AluOpType.mult,
                    )

                # Rotated output: dram row g*CPG*P + c*P + p' <- ot[p'+1, c, :]
                nc.gpsimd.dma_start(
                    out_rot[0 : P - 1, g * CPG : (g + 1) * CPG, :], ot[1:P, :, :]
                )
                nc.gpsimd.dma_start(
                    out_rot[P - 1 : P, g * CPG : (g + 1) * CPG, :], ot[0:1, :, :]
                )

    nc.compile()
    return nc


_NC_CACHE = None
last_results = None  # BassKernelResults of the most recent run (for test harness)


def kernel(inputs: np.ndarray) -> np.ndarray:
    global _NC_CACHE, last_results
    if _NC_CACHE is None:
        _NC_CACHE = _build()
    nc = _NC_CACHE
    x = np.ascontiguousarray(np.asarray(inputs, dtype=np.float32))
    assert x.shape == (B, T, F), x.shape
    in_maps = [{"x": x[b]} for b in range(B)]
    res = run_bass_kernel_spmd(nc, in_maps, core_ids=list(range(B)))
    last_results = res
    return np.stack([r["out"] for r in res.results], axis=0)


# revision 9
# speedup vs baseline: 1.0219x; 1.0219x over previous
"""Cumulative mean along T (running mean) for input [8, 4096, 1024] f32.

out[b, t, f] = mean(x[b, :t+1, f])

Pure data parallel over batch: 8 cores, one batch element each.
Per core, blocked prefix-sum along T in 128-row blocks:

  - Rotated-output triangular matmul per block (stationary col 0 = ones,
    col t>=1 ones for s <= t-1): PSUM partition 0 = block total (a legal AP
    base), partitions 1..127 = local prefixes of rows 0..126.
  - Main matmuls are carry-free and independent. They run in float32r
    (full-rate single-pass fp32): the input DRAM tensor is declared f32r
    (identical bits; numpy side stays float32).
  - Running carry: [1, F] SBUF row, carry_{i+1} = carry_i + psum_i[0:1]
    (one small DVE add per block - the only serial dependency).
  - Carry applied for blocks i>0 by a K=1 ones-broadcast f32r matmul
    accumulating into the same PSUM bank before the scale.
  - Per-row 1/(t+1) scale on the Scalar engine (Identity activation with a
    rotated per-partition reciprocal scale column), freeing VectorE for the
    carry chain.

DMA: inputs via HWDGE (nc.sync) - spreads DRAM->SBUF across all 16 SDMA
engines with no Q7 cost; outputs via SWDGE (nc.gpsimd) - the only path that
spreads SBUF->DRAM - batched 4 blocks (~2 MiB) per dma_start.
"""

import numpy as np

import concourse.bacc as bacc
import concourse.tile as tile
from concourse import mybir
from concourse.bass_utils import run_bass_kernel_spmd

B, T, F = 8, 4096, 1024
P = 128
NBLK = T // P  # 32
FH = 512       # one PSUM bank of f32
NHALF = F // FH
CPG = 4        # blocks per DMA group

F32 = mybir.dt.float32
F32R = mybir.dt.float32r


def _build():
    nc = bacc.Bacc(None, target_bir_lowering=False)
    x_dram = nc.dram_tensor("x", [T, F], F32R, kind="ExternalInput")
    out_dram = nc.dram_tensor("out", [T, F], F32, kind="ExternalOutput")

    # Rotated prefix-sum stationary: col 0 all ones, col t>=1 strict-upper.
    lt_np = np.triu(np.ones((P, P), dtype=np.float32), 1)
    lt_np[:, 0] = 1.0
    # psum partition p of block i holds output row r: p=0 -> r=i*128+127,
    # p>=1 -> r=i*128+p-1. scale = 1/(r+1).
    rows = np.arange(T, dtype=np.int64).reshape(NBLK, P)
    r_of_p = np.empty((NBLK, P), dtype=np.int64)
    r_of_p[:, 0] = rows[:, P - 1]
    r_of_p[:, 1:] = rows[:, : P - 1]
    recip_np = np.ascontiguousarray((1.0 / (r_of_p.T + 1.0)).astype(np.float32))
    lt_dram = nc.inline_tensor(lt_np, "lt_const")
    recip_dram = nc.inline_tensor(recip_np, "recip_const")

    x_rot = x_dram.rearrange("(n p) f -> p n f", p=P)
    out_rot = out_dram.rearrange("(n p) f -> p n f", p=P)

    with tile.TileContext(nc) as tc:
        with (
            tc.tile_pool(name="const", bufs=1) as cpool,
            tc.tile_pool(name="xin", bufs=3) as xpool,
            tc.tile_pool(name="xout", bufs=3) as opool,
            tc.tile_pool(name="run", bufs=4) as rpool,
            tc.tile_pool(name="psum", bufs=4, space="PSUM") as ppool,
        ):
            lt_f32 = cpool.tile([P, P], F32)
            nc.sync.dma_start(lt_f32[:], lt_dram[:])
            lt = cpool.tile([P, P], F32R)
            nc.vector.tensor_copy(lt[:], lt_f32[:])
            recip = cpool.tile([P, NBLK], F32)
            nc.sync.dma_start(recip[:], recip_dram[:])
            ones1_f32 = cpool.tile([1, P], F32)
            nc.vector.memset(ones1_f32[:], 1.0)
            ones1 = cpool.tile([1, P], F32R)
            nc.vector.tensor_copy(ones1[:], ones1_f32[:])

            carry = None  # [1, F] f32r SBUF row: sum of all raw blocks < i
            for g in range(NBLK // CPG):
                xt = xpool.tile([P, CPG, F], F32R)
                nc.sync.dma_start(xt[:], x_rot[:, g * CPG : (g + 1) * CPG, :])
                ot = opool.tile([P, CPG, F], F32)

                psums = []
                carries = []
                # Main matmuls (carry-free, independent) + the carry chain.
                for c in range(CPG):
                    i = g * CPG + c
                    ps = ppool.tile([P, F], F32)
                    psums.append(ps)
                    carries.append(carry)
                    for h in range(NHALF):
                        nc.tensor.matmul(
                            ps[:, h * FH : (h + 1) * FH],
                            lt[:],
                            xt[:, c, h * FH : (h + 1) * FH],
                            start=True,
                            stop=(i == 0),
                        )
                    # Chain hop: next carry = carry + this block's raw total
                    # (psum row 0 before the broadcast matmul adds carry).
                    if i < NBLK - 1:
                        new_carry = rpool.tile([1, F], F32R)
                        if carry is None:
                            nc.vector.tensor_copy(new_carry[:], ps[0:1, :])
                        else:
                            nc.vector.tensor_tensor(
                                new_carry[:],
                                carry[:].bitcast(F32),
                                ps[0:1, :],
                                mybir.AluOpType.add,
                            )
                        carry = new_carry

                # Carry broadcast matmuls (K=1 ones, f32r full-rate).
                for c in range(CPG):
                    i = g * CPG + c
                    if carries[c] is None:
                        continue
                    ps = psums[c]
                    for h in range(NHALF):
                        nc.tensor.matmul(
                            ps[:, h * FH : (h + 1) * FH],
                            ones1[:],
                            carries[c][:, h * FH : (h + 1) * FH],
                            start=False,
                            stop=True,
                        )

                # Scale on the Scalar engine: out = Identity(recip * psum).
                for c in range(CPG):
                    i = g * CPG + c
                    nc.scalar.activation(
                        ot[:, c, :],
                        psums[c][:],
                        mybir.ActivationFunctionType.Identity,
                        scale=recip[:, i : i + 1],
                    )

                # Rotated output: dram row g*CPG*P + c*P + p' <- ot[p'+1, c, :]
                nc.gpsimd.dma_start(
                    out_rot[0 : P - 1, g * CPG : (g + 1) * CPG, :], ot[1:P, :, :]
                )
                nc.gpsimd.dma_start(
                    out_rot[P - 1 : P, g * CPG : (g + 1) * CPG, :], ot[0:1, :, :]
                )

    nc.compile()
    return nc


_NC_CACHE = None
last_results = None  # BassKernelResults of the most recent run (for test harness)


def kernel(inputs: np.ndarray) -> np.ndarray:
    global _NC_CACHE, last_results
    if _NC_CACHE is None:
        _NC_CACHE = _build()
    nc = _NC_CACHE
    x = np.ascontiguousarray(np.asarray(inputs, dtype=np.float32))
    assert x.shape == (B, T, F), x.shape
    in_maps = [{"x": x[b]} for b in range(B)]
    res = run_bass_kernel_spmd(nc, in_maps, core_ids=list(range(B)))
    last_results = res
    return np.stack([r["out"] for r in res.results], axis=0)


# revision 11
# speedup vs baseline: 3.3441x; 3.2724x over previous
"""Cumulative mean along T (running mean) for input [8, 4096, 1024] f32.

out[b, t, f] = mean(x[b, :t+1, f])

Pure data parallel over batch: 8 cores, one batch element each.
Per core, blocked prefix-sum along T in 128-row blocks (all matmuls f32r =
full-rate single-pass fp32; the input DRAM tensor is declared f32r, which is
bit-identical):

  - main matmul per block: unrotated triangular ones stationary ->
    psum[t] = local prefix(t). Independent across blocks.
  - block-total matmul per block: ones-column stationary [K=128, M=1] ->
    psB[0:1] = column sums (PSUM partition 0 = legal AP base).
  - carry chain (the only serial dependency): carry_{i+1} = carry_i +
    psB_i[0:1], one [1, F] DVE add per block (~1.2 us each).
  - carry applied for i>0 by a K=1 ones-broadcast matmul accumulating into
    the main PSUM bank.
  - per-row 1/(t+1) scale on the Scalar engine (Identity activation with a
    per-partition reciprocal column).

DMA (the memory-bound axis): one 2 MiB HWDGE dma_start per 4-block group in
each direction, full 128-partition APs with 4 KiB contiguous rows - measured
~390 GB/s reads and ~180+ GB/s writes. (Partition-offset/127-partition
output APs - from an earlier rotated design - collapsed write bandwidth to
~45 GB/s; keep output DMAs full-partition and unrotated.)
"""

import numpy as np

import concourse.bacc as bacc
import concourse.tile as tile
from concourse import mybir
from concourse.bass_utils import run_bass_kernel_spmd

B, T, F = 8, 4096, 1024
P = 128
NBLK = T // P  # 32
FH = 512       # one PSUM bank of f32
NHALF = F // FH
CPG = 4        # blocks per DMA group

F32 = mybir.dt.float32
F32R = mybir.dt.float32r


def _build():
    nc = bacc.Bacc(None, target_bir_lowering=False)
    x_dram = nc.dram_tensor("x", [T, F], F32R, kind="ExternalInput")
    out_dram = nc.dram_tensor("out", [T, F], F32, kind="ExternalOutput")

    lt_np = np.triu(np.ones((P, P), dtype=np.float32))  # lt[s,t]=1 for s<=t
    recip_np = np.ascontiguousarray(
        (1.0 / (np.arange(1, T + 1, dtype=np.float64))).astype(np.float32)
        .reshape(NBLK, P).T
    )  # [p, i] = 1/(i*128+p+1)
    lt_dram = nc.inline_tensor(lt_np, "lt_const")
    recip_dram = nc.inline_tensor(recip_np, "recip_const")

    x_rot = x_dram.rearrange("(n p) f -> p n f", p=P)
    out_rot = out_dram.rearrange("(n p) f -> p n f", p=P)

    with tile.TileContext(nc) as tc:
        with (
            tc.tile_pool(name="const", bufs=1) as cpool,
            tc.tile_pool(name="xin", bufs=3) as xpool,
            tc.tile_pool(name="xout", bufs=3) as opool,
            tc.tile_pool(name="run", bufs=4) as rpool,
            tc.tile_pool(name="psum", bufs=2, space="PSUM") as ppool,
            tc.tile_pool(name="psum_tot", bufs=2, space="PSUM") as tpool,
        ):
            lt_f32 = cpool.tile([P, P], F32)
            nc.sync.dma_start(lt_f32[:], lt_dram[:])
            lt = cpool.tile([P, P], F32R)
            nc.vector.tensor_copy(lt[:], lt_f32[:])
            recip = cpool.tile([P, NBLK], F32)
            nc.sync.dma_start(recip[:], recip_dram[:])
            ones_f32 = cpool.tile([P, 1], F32)
            nc.vector.memset(ones_f32[:], 1.0)
            onescol = cpool.tile([P, 1], F32R)
            nc.vector.tensor_copy(onescol[:], ones_f32[:])
            ones1_f32 = cpool.tile([1, P], F32)
            nc.vector.memset(ones1_f32[:], 1.0)
            ones1 = cpool.tile([1, P], F32R)
            nc.vector.tensor_copy(ones1[:], ones1_f32[:])

            carry = None  # [1, F] f32r SBUF row: sum of all blocks < i
            for g in range(NBLK // CPG):
                xt = xpool.tile([P, CPG, F], F32R)
                nc.sync.dma_start(xt[:], x_rot[:, g * CPG : (g + 1) * CPG, :])
                ot = opool.tile([P, CPG, F], F32)

                psums = []
                carries = []
                for c in range(CPG):
                    i = g * CPG + c
                    # Block totals to PSUM partition 0.
                    psb = tpool.tile([1, F], F32)
                    ps = ppool.tile([P, F], F32)
                    psums.append(ps)
                    carries.append(carry)
                    for h in range(NHALF):
                        hs = slice(h * FH, (h + 1) * FH)
                        nc.tensor.matmul(
                            psb[0:1, hs], onescol[:], xt[:, c, hs],
                            start=True, stop=True,
                        )
                        nc.tensor.matmul(
                            ps[:, hs], lt[:], xt[:, c, hs],
                            start=True, stop=(i == 0),
                        )
                    # Carry chain hop (VectorE).
                    if i < NBLK - 1:
                        new_carry = rpool.tile([1, F], F32R)
                        if carry is None:
                            nc.vector.tensor_copy(new_carry[:], psb[0:1, :])
                        else:
                            nc.vector.tensor_tensor(
                                new_carry[:], carry[:].bitcast(F32), psb[0:1, :],
                                mybir.AluOpType.add,
                            )
                        carry = new_carry

                # Carry broadcast matmuls (K=1 ones, f32r full-rate).
                for c in range(CPG):
                    if carries[c] is None:
                        continue
                    for h in range(NHALF):
                        hs = slice(h * FH, (h + 1) * FH)
                        nc.tensor.matmul(
                            psums[c][:, hs], ones1[:], carries[c][:, hs],
                            start=False, stop=True,
                        )

                # Scale on the Scalar engine: out = Identity(recip * psum).
                for c in range(CPG):
                    i = g * CPG + c
                    nc.scalar.activation(
                        ot[:, c, :], psums[c][:],
                        mybir.ActivationFunctionType.Identity,
                        scale=recip[:, i : i + 1],
                    )

                # One full-partition 2 MiB output DMA per group (HWDGE).
                nc.sync.dma_start(
                    out_rot[:, g * CPG : (g + 1) * CPG, :], ot[:, :, :]
                )

    nc.compile()
    return nc


_NC_CACHE = None
last_results = None  # BassKernelResults of the most recent run (for test harness)


def kernel(inputs: np.ndarray) -> np.ndarray:
    global _NC_CACHE, last_results
    if _NC_CACHE is None:
        _NC_CACHE = _build()
    nc = _NC_CACHE
    x = np.ascontiguousarray(np.asarray(inputs, dtype=np.float32))
    assert x.shape == (B, T, F), x.shape
    in_maps = [{"x": x[b]} for b in range(B)]
    res = run_bass_kernel_spmd(nc, in_maps, core_ids=list(range(B)))
    last_results = res
    return np.stack([r["out"] for r in res.results], axis=0)


# revision 13
# speedup vs baseline: 3.4705x; 1.0378x over previous
"""Cumulative mean along T (running mean) for input [8, 4096, 1024] f32.

out[b, t, f] = mean(x[b, :t+1, f])

Pure data parallel over batch: 8 cores, one batch element each.
Per core, blocked prefix-sum along T in 128-row blocks (all matmuls f32r =
full-rate single-pass fp32; the input DRAM tensor is declared f32r, which is
bit-identical):

  - main matmul per block: unrotated triangular ones stationary ->
    psum[t] = local prefix(t). Independent across blocks.
  - block-total matmul per block: ones-column stationary [K=128, M=1] ->
    psB[0:1] = column sums (PSUM partition 0 = legal AP base).
  - carry chain (the only serial dependency): carry_{i+1} = carry_i +
    psB_i[0:1], one [1, F] DVE add per block (~1.2 us each).
  - carry applied for i>0 by a K=1 ones-broadcast matmul accumulating into
    the main PSUM bank.
  - per-row 1/(t+1) scale on the Scalar engine (Identity activation with a
    per-partition reciprocal column).

DMA (the memory-bound axis): one 2 MiB HWDGE dma_start per 4-block group in
each direction, full 128-partition APs with 4 KiB contiguous rows - measured
~390 GB/s reads and ~180+ GB/s writes. (Partition-offset/127-partition
output APs - from an earlier rotated design - collapsed write bandwidth to
~45 GB/s; keep output DMAs full-partition and unrotated.)
"""

import numpy as np

import concourse.bacc as bacc
import concourse.tile as tile
from concourse import mybir
from concourse.bass_utils import run_bass_kernel_spmd

B, T, F = 8, 4096, 1024
P = 128
NBLK = T // P  # 32
FH = 512       # one PSUM bank of f32
NHALF = F // FH
CPG = 4        # blocks per DMA group

F32 = mybir.dt.float32
F32R = mybir.dt.float32r


def _build():
    nc = bacc.Bacc(None, target_bir_lowering=False)
    x_dram = nc.dram_tensor("x", [T, F], F32R, kind="ExternalInput")
    out_dram = nc.dram_tensor("out", [T, F], F32, kind="ExternalOutput")

    lt_np = np.triu(np.ones((P, P), dtype=np.float32))  # lt[s,t]=1 for s<=t
    recip_np = np.ascontiguousarray(
        (1.0 / (np.arange(1, T + 1, dtype=np.float64))).astype(np.float32)
        .reshape(NBLK, P).T
    )  # [p, i] = 1/(i*128+p+1)
    lt_dram = nc.inline_tensor(lt_np, "lt_const")
    recip_dram = nc.inline_tensor(recip_np, "recip_const")

    x_rot = x_dram.rearrange("(n p) f -> p n f", p=P)
    out_rot = out_dram.rearrange("(n p) f -> p n f", p=P)

    with tile.TileContext(nc) as tc:
        with (
            tc.tile_pool(name="const", bufs=1) as cpool,
            tc.tile_pool(name="xin", bufs=4) as xpool,
            tc.tile_pool(name="xout", bufs=3) as opool,
            tc.tile_pool(name="run", bufs=4) as rpool,
            tc.tile_pool(name="psum", bufs=2, space="PSUM") as ppool,
            tc.tile_pool(name="psum_tot", bufs=2, space="PSUM") as tpool,
        ):
            lt_f32 = cpool.tile([P, P], F32)
            nc.sync.dma_start(lt_f32[:], lt_dram[:])
            lt = cpool.tile([P, P], F32R)
            nc.vector.tensor_copy(lt[:], lt_f32[:])
            recip = cpool.tile([P, NBLK], F32)
            nc.sync.dma_start(recip[:], recip_dram[:])
            ones_f32 = cpool.tile([P, 1], F32)
            nc.vector.memset(ones_f32[:], 1.0)
            onescol = cpool.tile([P, 1], F32R)
            nc.vector.tensor_copy(onescol[:], ones_f32[:])
            ones1_f32 = cpool.tile([1, P], F32)
            nc.vector.memset(ones1_f32[:], 1.0)
            ones1 = cpool.tile([1, P], F32R)
            nc.vector.tensor_copy(ones1[:], ones1_f32[:])

            carry = None  # [1, F] f32r SBUF row: sum of all blocks < i
            for g in range(NBLK // CPG):
                xt = xpool.tile([P, CPG, F], F32R)
                nc.sync.dma_start(xt[:], x_rot[:, g * CPG : (g + 1) * CPG, :])
                ot = opool.tile([P, CPG, F], F32)

                psums = []
                carries = []
                for c in range(CPG):
                    i = g * CPG + c
                    # Block totals to PSUM partition 0.
                    psb = tpool.tile([1, F], F32)
                    ps = ppool.tile([P, F], F32)
                    psums.append(ps)
                    carries.append(carry)
                    for h in range(NHALF):
                        hs = slice(h * FH, (h + 1) * FH)
                        nc.tensor.matmul(
                            psb[0:1, hs], onescol[:], xt[:, c, hs],
                            start=True, stop=True,
                        )
                        nc.tensor.matmul(
                            ps[:, hs], lt[:], xt[:, c, hs],
                            start=True, stop=(i == 0),
                        )
                    # Carry chain hop (VectorE).
                    if i < NBLK - 1:
                        new_carry = rpool.tile([1, F], F32R)
                        if carry is None:
                            nc.vector.tensor_copy(new_carry[:], psb[0:1, :])
                        else:
                            nc.vector.tensor_tensor(
                                new_carry[:], carry[:].bitcast(F32), psb[0:1, :],
                                mybir.AluOpType.add,
                            )
                        carry = new_carry

                # Carry broadcast matmuls (K=1 ones, f32r full-rate).
                for c in range(CPG):
                    if carries[c] is None:
                        continue
                    for h in range(NHALF):
                        hs = slice(h * FH, (h + 1) * FH)
                        nc.tensor.matmul(
                            psums[c][:, hs], ones1[:], carries[c][:, hs],
                            start=False, stop=True,
                        )

                # Scale, split across Scalar (h=0) and Vector (h=1) engines.
                for c in range(CPG):
                    i = g * CPG + c
                    nc.scalar.activation(
                        ot[:, c, 0:FH], psums[c][:, 0:FH],
                        mybir.ActivationFunctionType.Identity,
                        scale=recip[:, i : i + 1],
                    )
                    nc.vector.tensor_scalar(
                        ot[:, c, FH:F], psums[c][:, FH:F],
                        recip[:, i : i + 1], None, mybir.AluOpType.mult,
                    )

                # One full-partition 2 MiB output DMA per group, on the
                # Scalar HWDGE ring (inputs use the Sync ring).
                nc.scalar.dma_start(
                    out_rot[:, g * CPG : (g + 1) * CPG, :], ot[:, :, :]
                )

    nc.compile()
    return nc


_NC_CACHE = None
last_results = None  # BassKernelResults of the most recent run (for test harness)


def kernel(inputs: np.ndarray) -> np.ndarray:
    global _NC_CACHE, last_results
    if _NC_CACHE is None:
        _NC_CACHE = _build()
    nc = _NC_CACHE
    x = np.ascontiguousarray(np.asarray(inputs, dtype=np.float32))
    assert x.shape == (B, T, F), x.shape
    in_maps = [{"x": x[b]} for b in range(B)]
    res = run_bass_kernel_spmd(nc, in_maps, core_ids=list(range(B)))
    last_results = res
    return np.stack([r["out"] for r in res.results], axis=0)


# revision 14
# speedup vs baseline: 4.0072x; 1.1546x over previous
"""Cumulative mean along T (running mean) for input [8, 4096, 1024] f32.

out[b, t, f] = mean(x[b, :t+1, f])

Pure data parallel over batch: 8 cores, one batch element each.
Per core, blocked prefix-sum along T in 128-row blocks (all matmuls f32r =
full-rate single-pass fp32; the input DRAM tensor is declared f32r, which is
bit-identical):

  - main matmul per block: triangular-ones stationary -> psum[t] = local
    prefix(t). Independent across blocks, unrotated output rows.
  - carry chain (the only serial dependency): carry32_{i+1} = carry32_i +
    psum_i[96:128] - a [32, F] DVE add per block (legal 32-aligned AP base);
    only partition 31 (= psum row 127 = the block total) is meaningful, the
    other 31 partitions carry harmless finite junk.
  - carry applied for i>0 by a K=32 selector-broadcast matmul accumulating
    into the main PSUM bank: stationary sel[j, t] = 1 iff j == 31, so the PE
    array itself selects the carry row and broadcasts it to all 128 rows.
  - per-row 1/(t+1) scale split across Scalar (half 0) and Vector (half 1).

DMA (the memory-bound axis): one 2 MiB HWDGE dma_start per 4-block group in
each direction, full 128-partition APs with 4 KiB contiguous rows - measured
~390 GB/s reads and ~180+ GB/s writes. Inputs on the Sync ring, outputs on
the Scalar ring. (Partition-subset or partition-offset output APs collapse
write bandwidth to 45-70 GB/s - keep output DMAs full-partition.)
"""

import numpy as np

import concourse.bacc as bacc
import concourse.tile as tile
from concourse import mybir
from concourse.bass_utils import run_bass_kernel_spmd

B, T, F = 8, 4096, 1024
P = 128
NBLK = T // P  # 32
FH = 512       # one PSUM bank of f32
NHALF = F // FH
CPG = 4        # blocks per DMA group

F32 = mybir.dt.float32
F32R = mybir.dt.float32r


def _build():
    nc = bacc.Bacc(None, target_bir_lowering=False)
    x_dram = nc.dram_tensor("x", [T, F], F32R, kind="ExternalInput")
    out_dram = nc.dram_tensor("out", [T, F], F32, kind="ExternalOutput")

    lt_np = np.triu(np.ones((P, P), dtype=np.float32))  # lt[s,t]=1 for s<=t
    sel_np = np.zeros((32, P), dtype=np.float32)        # selects carry row 31
    sel_np[31, :] = 1.0
    recip_np = np.ascontiguousarray(
        (1.0 / (np.arange(1, T + 1, dtype=np.float64))).astype(np.float32)
        .reshape(NBLK, P).T
    )  # [p, i] = 1/(i*128+p+1)
    lt_dram = nc.inline_tensor(lt_np, "lt_const")
    sel_dram = nc.inline_tensor(sel_np, "sel_const")
    recip_dram = nc.inline_tensor(recip_np, "recip_const")

    x_rot = x_dram.rearrange("(n p) f -> p n f", p=P)
    out_rot = out_dram.rearrange("(n p) f -> p n f", p=P)

    with tile.TileContext(nc) as tc:
        with (
            tc.tile_pool(name="const", bufs=1) as cpool,
            tc.tile_pool(name="xin", bufs=4) as xpool,
            tc.tile_pool(name="xout", bufs=3) as opool,
            tc.tile_pool(name="run", bufs=4) as rpool,
            tc.tile_pool(name="psum", bufs=4, space="PSUM") as ppool,
        ):
            lt_f32 = cpool.tile([P, P], F32)
            nc.sync.dma_start(lt_f32[:], lt_dram[:])
            lt = cpool.tile([P, P], F32R)
            nc.vector.tensor_copy(lt[:], lt_f32[:])
            sel_f32 = cpool.tile([32, P], F32)
            nc.sync.dma_start(sel_f32[:], sel_dram[:])
            sel = cpool.tile([32, P], F32R)
            nc.vector.tensor_copy(sel[:], sel_f32[:])
            recip = cpool.tile([P, NBLK], F32)
            nc.sync.dma_start(recip[:], recip_dram[:])

            carry = None  # [32, F] f32r; partition 31 = sum of blocks < i
            for g in range(NBLK // CPG):
                xt = xpool.tile([P, CPG, F], F32R)
                nc.sync.dma_start(xt[:], x_rot[:, g * CPG : (g + 1) * CPG, :])
                ot = opool.tile([P, CPG, F], F32)

                psums = []
                carries = []
                for c in range(CPG):
                    i = g * CPG + c
                    ps = ppool.tile([P, F], F32)
                    psums.append(ps)
                    carries.append(carry)
                    for h in range(NHALF):
                        hs = slice(h * FH, (h + 1) * FH)
                        nc.tensor.matmul(
                            ps[:, hs], lt[:], xt[:, c, hs],
                            start=True, stop=(i == 0),
                        )
                    # Carry chain hop (VectorE), reading local prefix rows
                    # 96..127 before the broadcast matmul rewrites the bank.
                    if i < NBLK - 1:
                        new_carry = rpool.tile([32, F], F32R)
                        if carry is None:
                            nc.vector.tensor_copy(new_carry[:], ps[96:P, :])
                        else:
                            nc.vector.tensor_tensor(
                                new_carry[:], carry[:].bitcast(F32), ps[96:P, :],
                                mybir.AluOpType.add,
                            )
                        carry = new_carry

                # Carry broadcast matmuls (K=32 selector, f32r full-rate).
                for c in range(CPG):
                    if carries[c] is None:
                        continue
                    for h in range(NHALF):
                        hs = slice(h * FH, (h + 1) * FH)
                        nc.tensor.matmul(
                            psums[c][:, hs], sel[:], carries[c][:, hs],
                            start=False, stop=True,
                        )

                # Scale, split across Scalar (h=0) and Vector (h=1) engines.
                for c in range(CPG):
                    i = g * CPG + c
                    nc.scalar.activation(
                        ot[:, c, 0:FH], psums[c][:, 0:FH],
                        mybir.ActivationFunctionType.Identity,
                        scale=recip[:, i : i + 1],
                    )
                    nc.vector.tensor_scalar(
                        ot[:, c, FH:F], psums[c][:, FH:F],
                        recip[:, i : i + 1], None, mybir.AluOpType.mult,
                    )

                # One full-partition 2 MiB output DMA per group, on the
                # Scalar HWDGE ring (inputs use the Sync ring).
                nc.scalar.dma_start(
                    out_rot[:, g * CPG : (g + 1) * CPG, :], ot[:, :, :]
                )

    nc.compile()
    return nc


_NC_CACHE = None
last_results = None  # BassKernelResults of the most recent run (for test harness)


def kernel(inputs: np.ndarray) -> np.ndarray:
    global _NC_CACHE, last_results
    if _NC_CACHE is None:
        _NC_CACHE = _build()
    nc = _NC_CACHE
    x = np.ascontiguousarray(np.asarray(inputs, dtype=np.float32))
    assert x.shape == (B, T, F), x.shape
    in_maps = [{"x": x[b]} for b in range(B)]
    res = run_bass_kernel_spmd(nc, in_maps, core_ids=list(range(B)))
    last_results = res
    return np.stack([r["out"] for r in res.results], axis=0)


# revision 15
# speedup vs baseline: 4.5223x; 1.1286x over previous
"""Cumulative mean along T (running mean) for input [8, 4096, 1024] f32.

out[b, t, f] = mean(x[b, :t+1, f])

Pure data parallel over batch: 8 cores, one batch element each.
Per core, blocked prefix-sum along T in 128-row blocks (all matmuls f32r =
full-rate single-pass fp32; the input DRAM tensor is declared f32r, which is
bit-identical):

  - main matmul per block: triangular-ones stationary -> psum[t] = local
    prefix(t). Independent across blocks, unrotated output rows.
  - carry chain (the only serial dependency): carry32_{i+1} = carry32_i +
    psum_i[96:128] - a [32, F] DVE add per block (legal 32-aligned AP base);
    only partition 31 (= psum row 127 = the block total) is meaningful, the
    other 31 partitions carry harmless finite junk.
  - carry applied for i>0 by a K=32 selector-broadcast matmul accumulating
    into the main PSUM bank: stationary sel[j, t] = 1 iff j == 31, so the PE
    array itself selects the carry row and broadcasts it to all 128 rows.
  - per-row 1/(t+1) scale split across Scalar (half 0) and Vector (half 1).

DMA (the memory-bound axis): one 2 MiB HWDGE dma_start per 4-block group in
each direction, full 128-partition APs with 4 KiB contiguous rows - measured
~390 GB/s reads and ~180+ GB/s writes. Inputs on the Sync ring, outputs on
the Scalar ring. (Partition-subset or partition-offset output APs collapse
write bandwidth to 45-70 GB/s - keep output DMAs full-partition.)
"""

import numpy as np

import concourse.bacc as bacc
import concourse.tile as tile
from concourse import mybir
from concourse.bass_utils import run_bass_kernel_spmd

B, T, F = 8, 4096, 1024
P = 128
NBLK = T // P  # 32
FH = 512       # one PSUM bank of f32
NHALF = F // FH
CPG = 4        # blocks per DMA group

F32 = mybir.dt.float32
F32R = mybir.dt.float32r


def _build():
    nc = bacc.Bacc(None, target_bir_lowering=False)
    x_dram = nc.dram_tensor("x", [T, F], F32R, kind="ExternalInput")
    out_dram = nc.dram_tensor("out", [T, F], F32, kind="ExternalOutput")

    lt_np = np.triu(np.ones((P, P), dtype=np.float32))  # lt[s,t]=1 for s<=t
    sel_np = np.zeros((32, P), dtype=np.float32)        # selects carry row 31
    sel_np[31, :] = 1.0
    recip_np = np.ascontiguousarray(
        (1.0 / (np.arange(1, T + 1, dtype=np.float64))).astype(np.float32)
        .reshape(NBLK, P).T
    )  # [p, i] = 1/(i*128+p+1)
    lt_dram = nc.inline_tensor(lt_np, "lt_const")
    sel_dram = nc.inline_tensor(sel_np, "sel_const")
    recip_dram = nc.inline_tensor(recip_np, "recip_const")

    x_rot = x_dram.rearrange("(n p) f -> p n f", p=P)
    out_rot = out_dram.rearrange("(n p) f -> p n f", p=P)

    with tile.TileContext(nc) as tc:
        with (
            tc.tile_pool(name="const", bufs=1) as cpool,
            tc.tile_pool(name="xin", bufs=4) as xpool,
            tc.tile_pool(name="xout", bufs=3) as opool,
            tc.tile_pool(name="run", bufs=4) as rpool,
            tc.tile_pool(name="psum", bufs=4, space="PSUM") as ppool,
        ):
            lt_f32 = cpool.tile([P, P], F32)
            nc.sync.dma_start(lt_f32[:], lt_dram[:])
            lt = cpool.tile([P, P], F32R)
            nc.vector.tensor_copy(lt[:], lt_f32[:])
            sel_f32 = cpool.tile([32, P], F32)
            nc.sync.dma_start(sel_f32[:], sel_dram[:])
            sel = cpool.tile([32, P], F32R)
            nc.vector.tensor_copy(sel[:], sel_f32[:])
            recip = cpool.tile([P, NBLK], F32)
            nc.sync.dma_start(recip[:], recip_dram[:])

            carry = None  # [32, F] f32r; partition 31 = sum of blocks < i
            for g in range(NBLK // CPG):
                xt = xpool.tile([P, CPG, F], F32R)
                nc.sync.dma_start(xt[:], x_rot[:, g * CPG : (g + 1) * CPG, :])
                ot = opool.tile([P, CPG, F], F32)

                psums = []
                carries = []
                for c in range(CPG):
                    i = g * CPG + c
                    ps = ppool.tile([P, F], F32)
                    psums.append(ps)
                    carries.append(carry)
                    for h in range(NHALF):
                        hs = slice(h * FH, (h + 1) * FH)
                        nc.tensor.matmul(
                            ps[:, hs], lt[:], xt[:, c, hs],
                            start=True, stop=(i == 0),
                        )
                    # Carry chain hop (VectorE), reading local prefix rows
                    # 96..127 before the broadcast matmul rewrites the bank.
                    if i < NBLK - 1:
                        new_carry = rpool.tile([32, F], F32R)
                        for h in range(NHALF):
                            hs = slice(h * FH, (h + 1) * FH)
                            if carry is None:
                                nc.vector.tensor_copy(
                                    new_carry[:, hs], ps[96:P, hs]
                                )
                            else:
                                nc.vector.tensor_tensor(
                                    new_carry[:, hs],
                                    carry[:, hs].bitcast(F32),
                                    ps[96:P, hs],
                                    mybir.AluOpType.add,
                                )
                        carry = new_carry

                # Carry broadcast matmuls (K=32 selector, f32r full-rate).
                for c in range(CPG):
                    if carries[c] is None:
                        continue
                    for h in range(NHALF):
                        hs = slice(h * FH, (h + 1) * FH)
                        nc.tensor.matmul(
                            psums[c][:, hs], sel[:], carries[c][:, hs],
                            start=False, stop=True,
                        )

                # Scale, split across Scalar (h=0) and Vector (h=1) engines.
                for c in range(CPG):
                    i = g * CPG + c
                    nc.scalar.activation(
                        ot[:, c, 0:FH], psums[c][:, 0:FH],
                        mybir.ActivationFunctionType.Identity,
                        scale=recip[:, i : i + 1],
                    )
                    nc.vector.tensor_scalar(
                        ot[:, c, FH:F], psums[c][:, FH:F],
                        recip[:, i : i + 1], None, mybir.AluOpType.mult,
                    )

                # One full-partition 2 MiB output DMA per group, on the
                # Scalar HWDGE ring (inputs use the Sync ring).
                nc.scalar.dma_start(
                    out_rot[:, g * CPG : (g + 1) * CPG, :], ot[:, :, :]
                )

    nc.compile()
    return nc


_NC_CACHE = None
last_results = None  # BassKernelResults of the most recent run (for test harness)


def kernel(inputs: np.ndarray) -> np.ndarray:
    global _NC_CACHE, last_results
    if _NC_CACHE is None:
        _NC_CACHE = _build()
    nc = _NC_CACHE
    x = np.ascontiguousarray(np.asarray(inputs, dtype=np.float32))
    assert x.shape == (B, T, F), x.shape
    in_maps = [{"x": x[b]} for b in range(B)]
    res = run_bass_kernel_spmd(nc, in_maps, core_ids=list(range(B)))
    last_results = res
    return np.stack([r["out"] for r in res.results], axis=0)
